# revision 16
# baseline (speedup 1.0000x reference)
"""Trainium2 Bass kernel for the FCBlock weight-transform + matmul problem.

Math (per reference):
    W_i = per-head 3x3 conv over W.reshape(4, 1024, 4096) + conv_b
          + sigmoid(sk_wt) * W            (per-head scalars)
    out  = inp @ W_i.T                    (inp: [2, 2048, 4096])

Strategy: tensor-parallel shard of W along fout across 8 NeuronCores
(512 rows each).  The host pre-slices W^T with the conv halo (zero-padded
at head boundaries and fin edges), packs it into the SBUF staging layout,
builds the tiny banded conv matrices from conv_w^T/sigmoid(sk_wt), and
pre-casts everything to bf16.  On each core:
  - stage the whole W^T shard in SBUF (big DMAs issued ahead of the input
    transposes on the same HWDGE ring so they are not starved),
  - run the weight transform as PE band-matmuls accumulating in PSUM,
    which directly yields W_i^T (fin on partitions) - no transposes;
    the conv bias is added during the PSUM->SBUF copy,
  - stream inp via X-bar DMA-transpose (bf16) directly into fin-major
    layout, and run the main matmul in bf16 with fp32 PSUM accumulation.
Output is sharded on fout; the host concatenates.
"""

import numpy as np

import concourse.mybir as mybir
import concourse.tile as tile
from concourse import bacc
from concourse.bass_utils import run_bass_kernel_spmd

F32 = mybir.dt.float32
BF16 = mybir.dt.bfloat16

NCORES = 8
NUM_HEADS = 4
TOK = 4096          # 2 * 2048 tokens
FIN = 4096
FOUT = 4096
FSH = FOUT // NCORES  # 512 fout rows per core
SUP = 512           # token superblock (one transpose-DMA each)


def build_program(tok=TOK, fin=FIN):
    """Build the per-core SPMD program.

    tok/fin are parameters so a mini variant can be compiled quickly for
    validation; the graded path always uses the full sizes.
    """
    assert tok % SUP == 0 and fin % 128 == 0
    n_sup = tok // SUP           # 512-token superblocks
    n_k = fin // 128             # 128-deep contraction blocks / T windows

    nc = bacc.Bacc(None, target_bir_lowering=False)

    xb = nc.declare_dram_parameter("xb", [tok, fin], BF16, isOutput=False)
    wts = nc.declare_dram_parameter("wts", [128, n_k, FSH + 2], BF16,
                                    isOutput=False)
    hts = nc.declare_dram_parameter("hts", [6, n_k, FSH], BF16,
                                    isOutput=False)
    cbnd = nc.declare_dram_parameter("cbnd", [128, 4, 128], BF16,
                                     isOutput=False)
    cb = nc.declare_dram_parameter("cb", [128, 1], F32, isOutput=False)
    out = nc.declare_dram_parameter("o", [tok, FSH], F32, isOutput=True)

    with tile.TileContext(nc) as tc:
        with (
            tc.tile_pool(name="const", bufs=1) as const,
            tc.tile_pool(name="wtpool", bufs=1) as wtpool,
            tc.tile_pool(name="xt", bufs=2) as xtp,
            tc.tile_pool(name="osb", bufs=6) as osbp,
            tc.tile_pool(name="ps", bufs=8, space="PSUM") as ps,
        ):
            # ---- input staging: consts + W^T slab, all on the sync ring in
            # need-order (the input transposes queue strictly after -- FIFO
            # per ring; do NOT split across rings: the 8 shared DMA sem
            # lanes can block a staging DMA behind a 15us transpose)
            wst = const.tile([128, n_k, FSH + 2], BF16, tag="wst")
            hst = const.tile([6, n_k, FSH], BF16, tag="hst")
            cbnd_sb = const.tile([128, 4, 128], BF16)
            cb_sb = const.tile([128, 1], F32)
            n_chunk = max(1, n_k // 4)
            nc.sync.dma_start(out=wst[:, 0:n_chunk, :],
                              in_=wts[:, 0:n_chunk, :])
            nc.sync.dma_start(out=cbnd_sb[:], in_=cbnd[:])
            nc.sync.dma_start(out=cb_sb[:], in_=cb[:])
            nc.sync.dma_start(out=hst[:], in_=hts[:])
            for g in range(n_chunk, n_k, n_chunk):
                nc.sync.dma_start(out=wst[:, g:g + n_chunk, :],
                                  in_=wts[:, g:g + n_chunk, :])

            wt = wtpool.tile([128, n_k, FSH], BF16)        # W_i^T, fin-major

            # ---- phase T: weight transform straight into W_i^T ------------
            # window groups with tap-outer ordering: amortizes stationary
            # switches (the [6,...] halo tiles break LDWEIGHTS pull-ahead)
            nq = 8
            for q in range(0, n_k, nq):
                pws = [ps.tile([128, FSH], F32, tag="ps", name=f"pw{q}_{j}")
                       for j in range(min(nq, n_k - q))]
                for a in range(3):
                    for j, pw in enumerate(pws):
                        nc.tensor.matmul(
                            pw[:], cbnd_sb[:, a, :],
                            wst[:, q + j, a:a + FSH],
                            start=(a == 0), stop=False)
                for j, pw in enumerate(pws):
                    nc.tensor.matmul(pw[:], cbnd_sb[0:6, 3, :],
                                     hst[:, q + j, :],
                                     start=False, stop=True)
                for j, pw in enumerate(pws):
                    i = q + j
                    # PSUM -> SBUF with conv-bias add, cast to bf16
                    if j % 2 == 0:
                        nc.scalar.add(wt[:, i, :], pw[:], cb_sb[:, 0:1])
                    else:
                        nc.vector.tensor_scalar(
                            wt[:, i, :], pw[:], cb_sb[:, 0:1], None,
                            mybir.AluOpType.add)

            # ---- phase M: main matmul ---------------------------------
            for t in range(n_sup):
                xt = xtp.tile([128, n_k, SUP], BF16, tag="xt")
                nc.sync.dma_start(out=xt[:],
                                  in_=xb[SUP * t:SUP * t + SUP, :],
                                  transpose=True)
                for m in range(SUP // 128):
                    po = ps.tile([128, FSH], F32, tag="ps")
                    for k in range(n_k):
                        nc.tensor.matmul(po[:],
                                         xt[:, k, 128 * m:128 * m + 128],
                                         wt[:, k, :],
                                         start=(k == 0),
                                         stop=(k == n_k - 1))
                    ob = osbp.tile([128, FSH], F32, tag="ob")
                    if m % 2 == 0:
                        nc.scalar.copy(out=ob[:], in_=po[:])
                    else:
                        nc.vector.tensor_copy(out=ob[:], in_=po[:])
                    row0 = SUP * t + 128 * m
                    nc.scalar.dma_start(out=out[row0:row0 + 128, :],
                                        in_=ob[:])

    nc.compile()
    return nc


def shard_inputs(inp, W, conv_w, conv_b, sk_wt, fin=FIN):
    """Build the 8 per-core input maps: W^T fout-shard with conv halo,
    packed into the on-device staging layout, plus host-built band
    matrices (conv taps transposed, sigmoid residual folded in)."""
    bf = mybir.dt.np(BF16)
    tok = inp.size // fin
    xb = np.ascontiguousarray(
        inp.reshape(tok, fin)).astype(np.float32).astype(bf)
    W = np.asarray(W, dtype=np.float32)
    conv_w = np.asarray(conv_w, dtype=np.float32)
    hsz = W.shape[0] // NUM_HEADS  # rows per head
    n_k = fin // 128
    in_maps = []
    for c in range(NCORES):
        gr0 = c * FSH
        h = (gr0 // hsz) % NUM_HEADS
        # whal[R, C] = W[gr0-1+R, C-1], zero outside the head / fin range
        whal = np.zeros((FSH + 2, fin + 2), dtype=np.float32)
        lo = max(gr0 - 1, h * hsz)
        hi = min(gr0 + FSH + 1, (h + 1) * hsz)
        whal[lo - (gr0 - 1):hi - (gr0 - 1), 1:fin + 1] = W[lo:hi, :fin]
        # staged W^T: wts[k, i, c] = whal[c, 128i + k]
        wtslab = np.ascontiguousarray(whal.T)          # [fin+2, FSH+2]
        wts = np.ascontiguousarray(
            wtslab[:n_k * 128].reshape(n_k, 128, FSH + 2)
            .transpose(1, 0, 2))                       # [128, n_k, FSH+2]
        hts = np.empty((6, n_k, FSH), dtype=np.float32)
        for a in range(2):
            for b in range(3):
                hts[3 * a + b] = wtslab[128 + a:128 * n_k + 128 + a:128,
                                        b:b + FSH]
        # band matrices (conv taps transposed); sigmoid residual on the
        # (a=1, d=1) diagonal; halo matrix in cbnd[:, 3, :]
        cwt = conv_w[h].reshape(3, 3).T
        sig = float(1.0 / (1.0 + np.exp(-np.float64(
            np.asarray(sk_wt, dtype=np.float32)[h].reshape(())))))
        cbnd = np.zeros((128, 4, 128), dtype=np.float32)
        for a in range(3):
            for d in range(3):
                cbnd[:, a, :] += np.eye(128, k=-d, dtype=np.float32) \
                    * cwt[d, a]
        cbnd[:, 1, :] += np.eye(128, k=-1, dtype=np.float32) * sig
        h6 = np.zeros((6, 128), dtype=np.float32)
        for b in range(3):
            h6[b, 127] = cwt[1, b]
            h6[3 + b, 127] = cwt[2, b]
            h6[b, 126] = cwt[2, b]
        h6[1, 127] += sig
        cbnd[0:6, 3, :] = h6
        cbv = np.full((128, 1), np.float32(np.asarray(conv_b)[h]),
                      dtype=np.float32)
        in_maps.append({"xb": xb, "wts": wts.astype(bf),
                        "hts": hts.astype(bf),
                        "cbnd": cbnd.astype(bf), "cb": cbv})
    return in_maps


_PROGRAM_CACHE = {}


def _get_program(tok, fin):
    key = (tok, fin)
    if key not in _PROGRAM_CACHE:
        _PROGRAM_CACHE[key] = build_program(tok, fin)
    return _PROGRAM_CACHE[key]


def kernel(inp, W, conv_w, conv_b, sk_wt):
    nc = _get_program(TOK, FIN)
    in_maps = shard_inputs(inp, W, conv_w, conv_b, sk_wt)
    res = run_bass_kernel_spmd(nc, in_maps, list(range(NCORES)))
    shards = [res.results[c]["o"].reshape(2, TOK // 2, FSH)
              for c in range(NCORES)]
    return np.ascontiguousarray(
        np.concatenate(shards, axis=-1).astype(np.float32))


# revision 18
# speedup vs baseline: 1.0033x; 1.0033x over previous
"""Trainium2 Bass kernel for the FCBlock weight-transform + matmul problem.

Math (per reference):
    W_i = per-head 3x3 conv over W.reshape(4, 1024, 4096) + conv_b
          + sigmoid(sk_wt) * W            (per-head scalars)
    out  = inp @ W_i.T                    (inp: [2, 2048, 4096])

Strategy: tensor-parallel shard of W along fout across 8 NeuronCores
(512 rows each).  The host pre-slices W^T with the conv halo (zero-padded
at head boundaries and fin edges), packs it into the SBUF staging layout,
builds the tiny banded conv matrices from conv_w^T/sigmoid(sk_wt), and
pre-casts everything to bf16.  On each core:
  - stage the whole W^T shard in SBUF (big DMAs issued ahead of the input
    transposes on the same HWDGE ring so they are not starved),
  - run the weight transform as PE band-matmuls accumulating in PSUM,
    which directly yields W_i^T (fin on partitions) - no transposes;
    the conv bias is added during the PSUM->SBUF copy,
  - stream inp via X-bar DMA-transpose (bf16) directly into fin-major
    layout, and run the main matmul in bf16 with fp32 PSUM accumulation.
Output is sharded on fout; the host concatenates.
"""

import numpy as np

import concourse.mybir as mybir
import concourse.tile as tile
from concourse import bacc
from concourse.bass_utils import run_bass_kernel_spmd

F32 = mybir.dt.float32
BF16 = mybir.dt.bfloat16

NCORES = 8
NUM_HEADS = 4
TOK = 4096          # 2 * 2048 tokens
FIN = 4096
FOUT = 4096
FSH = FOUT // NCORES  # 512 fout rows per core
SUP = 512           # token superblock (one transpose-DMA each)


def build_program(tok=TOK, fin=FIN):
    """Build the per-core SPMD program.

    tok/fin are parameters so a mini variant can be compiled quickly for
    validation; the graded path always uses the full sizes.
    """
    assert tok % SUP == 0 and fin % 128 == 0
    n_sup = tok // SUP           # 512-token superblocks
    n_k = fin // 128             # 128-deep contraction blocks / T windows

    nc = bacc.Bacc(None, target_bir_lowering=False)

    xb = nc.declare_dram_parameter("xb", [tok, fin], BF16, isOutput=False)
    wts = nc.declare_dram_parameter("wts", [128, n_k, FSH + 2], BF16,
                                    isOutput=False)
    hts = nc.declare_dram_parameter("hts", [6, n_k, FSH], BF16,
                                    isOutput=False)
    cbnd = nc.declare_dram_parameter("cbnd", [128, 4, 128], BF16,
                                     isOutput=False)
    cb = nc.declare_dram_parameter("cb", [128, 1], F32, isOutput=False)
    out = nc.declare_dram_parameter("o", [tok, FSH], F32, isOutput=True)

    with tile.TileContext(nc) as tc:
        with (
            tc.tile_pool(name="const", bufs=1) as const,
            tc.tile_pool(name="wtpool", bufs=1) as wtpool,
            tc.tile_pool(name="xt", bufs=2) as xtp,
            tc.tile_pool(name="osb", bufs=6) as osbp,
            tc.tile_pool(name="ps", bufs=8, space="PSUM") as ps,
        ):
            # ---- input staging: consts + W^T slab, all on the sync ring in
            # need-order (the input transposes queue strictly after -- FIFO
            # per ring; do NOT split across rings: the 8 shared DMA sem
            # lanes can block a staging DMA behind a 15us transpose)
            wst = const.tile([128, n_k, FSH + 2], BF16, tag="wst")
            hst = const.tile([6, n_k, FSH], BF16, tag="hst")
            cbnd_sb = const.tile([128, 4, 128], BF16)
            cb_sb = const.tile([128, 1], F32)
            c0 = max(1, n_k // 8)
            nc.sync.dma_start(out=wst[:, 0:c0, :], in_=wts[:, 0:c0, :])
            nc.sync.dma_start(out=cbnd_sb[:], in_=cbnd[:])
            nc.sync.dma_start(out=cb_sb[:], in_=cb[:])
            nc.sync.dma_start(out=hst[:], in_=hts[:])
            nc.sync.dma_start(out=wst[:, c0:2 * c0, :],
                              in_=wts[:, c0:2 * c0, :])
            n_chunk = max(1, n_k // 4)
            for g in range(2 * c0, n_k, n_chunk):
                sz = min(n_chunk, n_k - g)
                nc.sync.dma_start(out=wst[:, g:g + sz, :],
                                  in_=wts[:, g:g + sz, :])

            wt = wtpool.tile([128, n_k, FSH], BF16)        # W_i^T, fin-major

            # ---- phase T: weight transform straight into W_i^T ------------
            # window groups with tap-outer ordering: amortizes stationary
            # switches (the [6,...] halo tiles break LDWEIGHTS pull-ahead)
            groups = []
            q = 0
            while q < n_k:
                nq = min(c0 if q < 2 * c0 else 8, n_k - q)
                groups.append((q, nq))
                q += nq
            for q, nq in groups:
                pws = [ps.tile([128, FSH], F32, tag="ps", name=f"pw{q}_{j}")
                       for j in range(nq)]
                for a in range(3):
                    for j, pw in enumerate(pws):
                        nc.tensor.matmul(
                            pw[:], cbnd_sb[:, a, :],
                            wst[:, q + j, a:a + FSH],
                            start=(a == 0), stop=False)
                for j, pw in enumerate(pws):
                    nc.tensor.matmul(pw[:], cbnd_sb[0:6, 3, :],
                                     hst[:, q + j, :],
                                     start=False, stop=True)
                for j, pw in enumerate(pws):
                    i = q + j
                    # PSUM -> SBUF with conv-bias add, cast to bf16
                    if j % 2 == 0:
                        nc.scalar.add(wt[:, i, :], pw[:], cb_sb[:, 0:1])
                    else:
                        nc.vector.tensor_scalar(
                            wt[:, i, :], pw[:], cb_sb[:, 0:1], None,
                            mybir.AluOpType.add)

            # ---- phase M: main matmul ---------------------------------
            for t in range(n_sup):
                xt = xtp.tile([128, n_k, SUP], BF16, tag="xt")
                nc.sync.dma_start(out=xt[:],
                                  in_=xb[SUP * t:SUP * t + SUP, :],
                                  transpose=True)
                for m in range(SUP // 128):
                    po = ps.tile([128, FSH], F32, tag="ps")
                    for k in range(n_k):
                        nc.tensor.matmul(po[:],
                                         xt[:, k, 128 * m:128 * m + 128],
                                         wt[:, k, :],
                                         start=(k == 0),
                                         stop=(k == n_k - 1))
                    ob = osbp.tile([128, FSH], F32, tag="ob")
                    if m % 2 == 0:
                        nc.scalar.copy(out=ob[:], in_=po[:])
                    else:
                        nc.vector.tensor_copy(out=ob[:], in_=po[:])
                    row0 = SUP * t + 128 * m
                    nc.scalar.dma_start(out=out[row0:row0 + 128, :],
                                        in_=ob[:])

    nc.compile()
    return nc


def shard_inputs(inp, W, conv_w, conv_b, sk_wt, fin=FIN):
    """Build the 8 per-core input maps: W^T fout-shard with conv halo,
    packed into the on-device staging layout, plus host-built band
    matrices (conv taps transposed, sigmoid residual folded in)."""
    bf = mybir.dt.np(BF16)
    tok = inp.size // fin
    xb = np.ascontiguousarray(
        inp.reshape(tok, fin)).astype(np.float32).astype(bf)
    W = np.asarray(W, dtype=np.float32)
    conv_w = np.asarray(conv_w, dtype=np.float32)
    hsz = W.shape[0] // NUM_HEADS  # rows per head
    n_k = fin // 128
    in_maps = []
    for c in range(NCORES):
        gr0 = c * FSH
        h = (gr0 // hsz) % NUM_HEADS
        # whal[R, C] = W[gr0-1+R, C-1], zero outside the head / fin range
        whal = np.zeros((FSH + 2, fin + 2), dtype=np.float32)
        lo = max(gr0 - 1, h * hsz)
        hi = min(gr0 + FSH + 1, (h + 1) * hsz)
        whal[lo - (gr0 - 1):hi - (gr0 - 1), 1:fin + 1] = W[lo:hi, :fin]
        # staged W^T: wts[k, i, c] = whal[c, 128i + k]
        wtslab = np.ascontiguousarray(whal.T)          # [fin+2, FSH+2]
        wts = np.ascontiguousarray(
            wtslab[:n_k * 128].reshape(n_k, 128, FSH + 2)
            .transpose(1, 0, 2))                       # [128, n_k, FSH+2]
        hts = np.empty((6, n_k, FSH), dtype=np.float32)
        for a in range(2):
            for b in range(3):
                hts[3 * a + b] = wtslab[128 + a:128 * n_k + 128 + a:128,
                                        b:b + FSH]
        # band matrices (conv taps transposed); sigmoid residual on the
        # (a=1, d=1) diagonal; halo matrix in cbnd[:, 3, :]
        cwt = conv_w[h].reshape(3, 3).T
        sig = float(1.0 / (1.0 + np.exp(-np.float64(
            np.asarray(sk_wt, dtype=np.float32)[h].reshape(())))))
        cbnd = np.zeros((128, 4, 128), dtype=np.float32)
        for a in range(3):
            for d in range(3):
                cbnd[:, a, :] += np.eye(128, k=-d, dtype=np.float32) \
                    * cwt[d, a]
        cbnd[:, 1, :] += np.eye(128, k=-1, dtype=np.float32) * sig
        h6 = np.zeros((6, 128), dtype=np.float32)
        for b in range(3):
            h6[b, 127] = cwt[1, b]
            h6[3 + b, 127] = cwt[2, b]
            h6[b, 126] = cwt[2, b]
        h6[1, 127] += sig
        cbnd[0:6, 3, :] = h6
        cbv = np.full((128, 1), np.float32(np.asarray(conv_b)[h]),
                      dtype=np.float32)
        in_maps.append({"xb": xb, "wts": wts.astype(bf),
                        "hts": hts.astype(bf),
                        "cbnd": cbnd.astype(bf), "cb": cbv})
    return in_maps


_PROGRAM_CACHE = {}


def _get_program(tok, fin):
    key = (tok, fin)
    if key not in _PROGRAM_CACHE:
        _PROGRAM_CACHE[key] = build_program(tok, fin)
    return _PROGRAM_CACHE[key]


def kernel(inp, W, conv_w, conv_b, sk_wt):
    nc = _get_program(TOK, FIN)
    in_maps = shard_inputs(inp, W, conv_w, conv_b, sk_wt)
    res = run_bass_kernel_spmd(nc, in_maps, list(range(NCORES)))
    shards = [res.results[c]["o"].reshape(2, TOK // 2, FSH)
              for c in range(NCORES)]
    return np.ascontiguousarray(
        np.concatenate(shards, axis=-1).astype(np.float32))


# revision 19
# speedup vs baseline: 1.0044x; 1.0011x over previous
"""Trainium2 Bass kernel for the FCBlock weight-transform + matmul problem.

Math (per reference):
    W_i = per-head 3x3 conv over W.reshape(4, 1024, 4096) + conv_b
          + sigmoid(sk_wt) * W            (per-head scalars)
    out  = inp @ W_i.T                    (inp: [2, 2048, 4096])

Strategy: tensor-parallel shard of W along fout across 8 NeuronCores
(512 rows each).  The host pre-slices W^T with the conv halo (zero-padded
at head boundaries and fin edges), packs it into the SBUF staging layout,
builds the tiny banded conv matrices from conv_w^T/sigmoid(sk_wt), and
pre-casts everything to bf16.  On each core:
  - stage the whole W^T shard in SBUF (big DMAs issued ahead of the input
    transposes on the same HWDGE ring so they are not starved),
  - run the weight transform as PE band-matmuls accumulating in PSUM,
    which directly yields W_i^T (fin on partitions) - no transposes;
    the conv bias is added during the PSUM->SBUF copy,
  - stream inp via X-bar DMA-transpose (bf16) directly into fin-major
    layout, and run the main matmul in bf16 with fp32 PSUM accumulation.
Output is sharded on fout; the host concatenates.
"""

import numpy as np

import concourse.mybir as mybir
import concourse.tile as tile
from concourse import bacc
from concourse.bass_utils import run_bass_kernel_spmd

F32 = mybir.dt.float32
BF16 = mybir.dt.bfloat16

NCORES = 8
NUM_HEADS = 4
TOK = 4096          # 2 * 2048 tokens
FIN = 4096
FOUT = 4096
FSH = FOUT // NCORES  # 512 fout rows per core
SUP = 512           # token superblock (one transpose-DMA each)


def build_program(tok=TOK, fin=FIN):
    """Build the per-core SPMD program.

    tok/fin are parameters so a mini variant can be compiled quickly for
    validation; the graded path always uses the full sizes.
    """
    assert tok % SUP == 0 and fin % 128 == 0
    n_sup = tok // SUP           # 512-token superblocks
    n_k = fin // 128             # 128-deep contraction blocks / T windows

    nc = bacc.Bacc(None, target_bir_lowering=False)

    xb = nc.declare_dram_parameter("xb", [tok, fin], BF16, isOutput=False)
    wts = nc.declare_dram_parameter("wts", [128, n_k, FSH + 2], BF16,
                                    isOutput=False)
    hts = nc.declare_dram_parameter("hts", [6, n_k, FSH], BF16,
                                    isOutput=False)
    cbnd = nc.declare_dram_parameter("cbnd", [128, 4, 128], BF16,
                                     isOutput=False)
    cb = nc.declare_dram_parameter("cb", [128, 1], F32, isOutput=False)
    out = nc.declare_dram_parameter("o", [tok, FSH], F32, isOutput=True)

    with tile.TileContext(nc) as tc:
        with (
            tc.tile_pool(name="const", bufs=1) as const,
            tc.tile_pool(name="wtpool", bufs=1) as wtpool,
            tc.tile_pool(name="xt", bufs=2) as xtp,
            tc.tile_pool(name="osb", bufs=6) as osbp,
            tc.tile_pool(name="ps", bufs=8, space="PSUM") as ps,
        ):
            # ---- input staging: consts + W^T slab, all on the sync ring in
            # need-order (the input transposes queue strictly after -- FIFO
            # per ring; do NOT split across rings: the 8 shared DMA sem
            # lanes can block a staging DMA behind a 15us transpose)
            wst = const.tile([128, n_k, FSH + 2], BF16, tag="wst")
            hst = const.tile([6, n_k, FSH], BF16, tag="hst")
            cbnd_sb = const.tile([128, 4, 128], BF16)
            cb_sb = const.tile([128, 1], F32)
            n_chunk = max(1, n_k // 4)
            nc.sync.dma_start(out=wst[:, 0:n_chunk, :],
                              in_=wts[:, 0:n_chunk, :])
            nc.sync.dma_start(out=cbnd_sb[:], in_=cbnd[:])
            nc.sync.dma_start(out=cb_sb[:], in_=cb[:])
            nc.sync.dma_start(out=hst[:], in_=hts[:])
            for g in range(n_chunk, n_k, n_chunk):
                nc.sync.dma_start(out=wst[:, g:g + n_chunk, :],
                                  in_=wts[:, g:g + n_chunk, :])

            wt = wtpool.tile([128, n_k, FSH], BF16)        # W_i^T, fin-major

            # ---- phase T: weight transform straight into W_i^T ------------
            # window groups with tap-outer ordering: amortizes stationary
            # switches (the [6,...] halo tiles break LDWEIGHTS pull-ahead)
            nq = 8
            for q in range(0, n_k, nq):
                pws = [ps.tile([128, FSH], F32, tag="ps", name=f"pw{q}_{j}")
                       for j in range(min(nq, n_k - q))]
                for a in range(3):
                    for j, pw in enumerate(pws):
                        nc.tensor.matmul(
                            pw[:], cbnd_sb[:, a, :],
                            wst[:, q + j, a:a + FSH],
                            start=(a == 0), stop=False)
                for j, pw in enumerate(pws):
                    nc.tensor.matmul(pw[:], cbnd_sb[0:6, 3, :],
                                     hst[:, q + j, :],
                                     start=False, stop=True)
                for j, pw in enumerate(pws):
                    i = q + j
                    # PSUM -> SBUF with conv-bias add, cast to bf16
                    if j % 2 == 0:
                        nc.scalar.add(wt[:, i, :], pw[:], cb_sb[:, 0:1])
                    else:
                        nc.vector.tensor_scalar(
                            wt[:, i, :], pw[:], cb_sb[:, 0:1], None,
                            mybir.AluOpType.add)

            # ---- phase M: main matmul ---------------------------------
            for t in range(n_sup):
                xt = xtp.tile([128, n_k, SUP], BF16, tag="xt")
                nc.sync.dma_start(out=xt[:],
                                  in_=xb[SUP * t:SUP * t + SUP, :],
                                  transpose=True)
                for m in range(SUP // 128):
                    po = ps.tile([128, FSH], F32, tag="ps")
                    for k in range(n_k):
                        nc.tensor.matmul(po[:],
                                         xt[:, k, 128 * m:128 * m + 128],
                                         wt[:, k, :],
                                         start=(k == 0),
                                         stop=(k == n_k - 1))
                    ob = osbp.tile([128, FSH], F32, tag="ob")
                    if m % 2 == 0:
                        nc.scalar.copy(out=ob[:], in_=po[:])
                    else:
                        nc.vector.tensor_copy(out=ob[:], in_=po[:])
                    row0 = SUP * t + 128 * m
                    nc.scalar.dma_start(out=out[row0:row0 + 128, :],
                                        in_=ob[:])

    nc.compile()
    return nc


def shard_inputs(inp, W, conv_w, conv_b, sk_wt, fin=FIN):
    """Build the 8 per-core input maps: W^T fout-shard with conv halo,
    packed into the on-device staging layout, plus host-built band
    matrices (conv taps transposed, sigmoid residual folded in)."""
    bf = mybir.dt.np(BF16)
    tok = inp.size // fin
    xb = np.ascontiguousarray(
        inp.reshape(tok, fin)).astype(np.float32).astype(bf)
    W = np.asarray(W, dtype=np.float32)
    conv_w = np.asarray(conv_w, dtype=np.float32)
    hsz = W.shape[0] // NUM_HEADS  # rows per head
    n_k = fin // 128
    in_maps = []
    for c in range(NCORES):
        gr0 = c * FSH
        h = (gr0 // hsz) % NUM_HEADS
        # whal[R, C] = W[gr0-1+R, C-1], zero outside the head / fin range
        whal = np.zeros((FSH + 2, fin + 2), dtype=np.float32)
        lo = max(gr0 - 1, h * hsz)
        hi = min(gr0 + FSH + 1, (h + 1) * hsz)
        whal[lo - (gr0 - 1):hi - (gr0 - 1), 1:fin + 1] = W[lo:hi, :fin]
        # staged W^T: wts[k, i, c] = whal[c, 128i + k]
        wtslab = np.ascontiguousarray(whal.T)          # [fin+2, FSH+2]
        wts = np.ascontiguousarray(
            wtslab[:n_k * 128].reshape(n_k, 128, FSH + 2)
            .transpose(1, 0, 2))                       # [128, n_k, FSH+2]
        hts = np.empty((6, n_k, FSH), dtype=np.float32)
        for a in range(2):
            for b in range(3):
                hts[3 * a + b] = wtslab[128 + a:128 * n_k + 128 + a:128,
                                        b:b + FSH]
        # band matrices (conv taps transposed); sigmoid residual on the
        # (a=1, d=1) diagonal; halo matrix in cbnd[:, 3, :]
        cwt = conv_w[h].reshape(3, 3).T
        sig = float(1.0 / (1.0 + np.exp(-np.float64(
            np.asarray(sk_wt, dtype=np.float32)[h].reshape(())))))
        cbnd = np.zeros((128, 4, 128), dtype=np.float32)
        for a in range(3):
            for d in range(3):
                cbnd[:, a, :] += np.eye(128, k=-d, dtype=np.float32) \
                    * cwt[d, a]
        cbnd[:, 1, :] += np.eye(128, k=-1, dtype=np.float32) * sig
        h6 = np.zeros((6, 128), dtype=np.float32)
        for b in range(3):
            h6[b, 127] = cwt[1, b]
            h6[3 + b, 127] = cwt[2, b]
            h6[b, 126] = cwt[2, b]
        h6[1, 127] += sig
        cbnd[0:6, 3, :] = h6
        cbv = np.full((128, 1), np.float32(np.asarray(conv_b)[h]),
                      dtype=np.float32)
        in_maps.append({"xb": xb, "wts": wts.astype(bf),
                        "hts": hts.astype(bf),
                        "cbnd": cbnd.astype(bf), "cb": cbv})
    return in_maps


_PROGRAM_CACHE = {}


def _get_program(tok, fin):
    key = (tok, fin)
    if key not in _PROGRAM_CACHE:
        _PROGRAM_CACHE[key] = build_program(tok, fin)
    return _PROGRAM_CACHE[key]


def kernel(inp, W, conv_w, conv_b, sk_wt):
    nc = _get_program(TOK, FIN)
    in_maps = shard_inputs(inp, W, conv_w, conv_b, sk_wt)
    res = run_bass_kernel_spmd(nc, in_maps, list(range(NCORES)))
    shards = [res.results[c]["o"].reshape(2, TOK // 2, FSH)
              for c in range(NCORES)]
    return np.ascontiguousarray(
        np.concatenate(shards, axis=-1).astype(np.float32))
